# revision 6
# baseline (speedup 1.0000x reference)
"""Local windowed multi-head attention on 8 TRN2 NeuronCores.

Sharding: core c = (b, g) with b = c // 2 (batch), g = c % 2 (head group of 8).
Each core computes qkv = x[b] @ w_qkv[:, head-group cols] and the windowed
attention for its 8 heads over the full sequence. Outputs are disjoint
column slices of the final (B, L, D) tensor -> no collectives.

Per-core kernel (Tile framework):
  phase 1 (per 512-seq chunk): load x, PE-transpose to xT (feat-major),
    GEMM qT/kT (feature-major) and v (seq-major, 65-col per-head layout with
    a ones column for softmax row sums).
  phase 2 (attention, per window x head): S^T = kT_slice.T @ qT_slice per
    key-window (keys on partitions), exp on ScalarE (scale folded in, no max
    subtraction -- scores are bounded), O = P @ [V|1] accumulated over key
    windows on PE; ones column yields softmax denominators; normalize with
    DVE reciprocal + tensor_scalar_mul.
"""

import numpy as np

import concourse.bass as bass
import concourse.bacc as bacc
import concourse.mybir as mybir
import concourse.tile as tile
from concourse.bass_utils import run_bass_kernel_spmd
from concourse.masks import make_identity

# Problem constants (hardcoded per spec)
B, L, D = 4, 4096, 1024
H, W, E = 16, 128, 64
HPC = H // 2          # heads per core = 8
F = HPC * E           # per-core feature cols = 512
NW = L // W           # 32 windows
CH = 512              # seq chunk = 4 windows
NCH = L // CH         # 8 chunks
WPC = CH // W         # 4 windows per chunk
KF = D // 128         # 8 contraction tiles
NF = F // 128         # 4 feature tiles
SCALE = float(E) ** -0.5

F32 = mybir.dt.float32
R = mybir.dt.float32r
EXP = mybir.ActivationFunctionType.Exp

_NC_CACHE = []


def _build_nc():
    nc = bacc.Bacc()
    x_d = nc.dram_tensor("x", [L, D], F32, kind="ExternalInput")
    wq_d = nc.dram_tensor("wq", [D, F], F32, kind="ExternalInput")
    wk_d = nc.dram_tensor("wk", [D, F], F32, kind="ExternalInput")
    wv_d = nc.dram_tensor("wv", [D, F], F32, kind="ExternalInput")
    out_d = nc.dram_tensor("out", [L, F], F32, kind="ExternalOutput")

    with tile.TileContext(nc) as tc:
        with (
            tc.tile_pool(name="wpool", bufs=8) as wpool,
            tc.tile_pool(name="cpool", bufs=1) as cpool,
            tc.tile_pool(name="xin", bufs=6) as xin_pool,
            tc.tile_pool(name="xt", bufs=12) as xt_pool,
            tc.tile_pool(name="qt", bufs=8) as qt_pool,
            tc.tile_pool(name="kt", bufs=16) as kt_pool,
            tc.tile_pool(name="vt", bufs=16) as vt_pool,
            tc.tile_pool(name="pt", bufs=3) as pt_pool,
            tc.tile_pool(name="osb", bufs=3) as osb_pool,
            tc.tile_pool(name="rcp", bufs=4) as rcp_pool,
            tc.tile_pool(name="tp_ps", bufs=2, space="PSUM") as tp_psum,
            tc.tile_pool(name="mm_ps", bufs=2, space="PSUM") as mm_psum,
            tc.tile_pool(name="st_ps", bufs=2, space="PSUM") as st_psum,
            tc.tile_pool(name="o_ps", bufs=2, space="PSUM") as o_psum,
        ):
            # --- persistent weights + identity ---
            wq_sb, wk_sb, wv_sb = [], [], []
            for kf in range(KF):
                wq_t = wpool.tile([128, F], R, name=f"wq{kf}", tag="wq")
                nc.sync.dma_start(wq_t, wq_d[kf * 128:(kf + 1) * 128, :].bitcast(R))
                wq_sb.append(wq_t)
                wk_t = wpool.tile([128, F], R, name=f"wk{kf}", tag="wk")
                nc.sync.dma_start(wk_t, wk_d[kf * 128:(kf + 1) * 128, :].bitcast(R))
                wk_sb.append(wk_t)
                wv_t = wpool.tile([128, F], R, name=f"wv{kf}", tag="wv")
                nc.sync.dma_start(wv_t, wv_d[kf * 128:(kf + 1) * 128, :].bitcast(R))
                wv_sb.append(wv_t)
            ident = cpool.tile([128, 128], F32, name="ident", tag="ident")
            make_identity(nc, ident)

            qts = {}  # chunk -> [NF tiles (128, CH)] feature-major q
            kts = {}  # chunk -> [NF tiles (128, CH)] feature-major k
            vts = {}  # chunk -> [WPC tiles (128, HPC*65)] seq-major v + ones col

            def phase1(c):
                xin = []
                for t in range(WPC):
                    xt_in = xin_pool.tile([128, D], F32, name=f"xin{c}_{t}",
                                          tag="xin")
                    r0 = c * CH + t * 128
                    nc.sync.dma_start(xt_in, x_d[r0:r0 + 128, :])
                    xin.append(xt_in)
                xTs = []
                for kf in range(KF):
                    tp = tp_psum.tile([128, CH], F32, name=f"tp{c}_{kf}",
                                      tag="tp")
                    for t in range(WPC):
                        nc.tensor.transpose(
                            tp[:, t * 128:(t + 1) * 128],
                            xin[t][:, kf * 128:(kf + 1) * 128],
                            ident,
                        )
                    xT = xt_pool.tile([128, CH], R, name=f"xT{c}_{kf}",
                                      tag="xt")
                    nc.vector.tensor_copy(xT, tp)
                    xTs.append(xT)
                # qT / kT GEMM (feature-major outputs)
                qts[c], kts[c] = [], []
                for nf in range(NF):
                    ps = mm_psum.tile([128, CH], F32, name=f"qps{c}_{nf}",
                                      tag="mm")
                    for kf in range(KF):
                        nc.tensor.matmul(
                            ps,
                            wq_sb[kf][:, nf * 128:(nf + 1) * 128],
                            xTs[kf],
                            start=(kf == 0), stop=(kf == KF - 1),
                        )
                    qt_t = qt_pool.tile([128, CH], R, name=f"qt{c}_{nf}",
                                        tag="qt")
                    nc.vector.tensor_copy(qt_t, ps)
                    qts[c].append(qt_t)
                for nf in range(NF):
                    ps = mm_psum.tile([128, CH], F32, name=f"kps{c}_{nf}",
                                      tag="mm")
                    for kf in range(KF):
                        nc.tensor.matmul(
                            ps,
                            wk_sb[kf][:, nf * 128:(nf + 1) * 128],
                            xTs[kf],
                            start=(kf == 0), stop=(kf == KF - 1),
                        )
                    kt_t = kt_pool.tile([128, CH], R, name=f"kt{c}_{nf}",
                                        tag="kt")
                    nc.vector.tensor_copy(kt_t, ps)
                    kts[c].append(kt_t)
                # v GEMM (seq-major, strided into 65-col per-head layout)
                vts[c] = []
                for st in range(WPC):
                    ps = mm_psum.tile([128, CH], F32, name=f"vps{c}_{st}",
                                      tag="mm")
                    for kf in range(KF):
                        nc.tensor.matmul(
                            ps,
                            xTs[kf][:, st * 128:(st + 1) * 128],
                            wv_sb[kf],
                            start=(kf == 0), stop=(kf == KF - 1),
                        )
                    vt_t = vt_pool.tile([128, HPC * 66], R,
                                        name=f"vt{c}_{st}", tag="vt")
                    v_view = vt_t.rearrange("p (h e) -> p h e", e=66)
                    nc.vector.tensor_copy(
                        v_view[:, :, 0:64],
                        ps.rearrange("p (h e) -> p h e", e=64),
                    )
                    nc.scalar.activation(
                        v_view[:, :, 64:66],
                        ps.rearrange("p (h e) -> p h e", e=64)[:, :, 0:2],
                        mybir.ActivationFunctionType.Copy,
                        bias=1.0, scale=0.0,
                    )
                    vts[c].append(vt_t)

            def attn(c):
                for wi in range(WPC):
                    w = c * WPC + wi
                    osb = osb_pool.tile([128, F], F32, name=f"osb{w}",
                                        tag="osb")
                    kws = [kw for kw in (w - 1, w, w + 1) if 0 <= kw < NW]
                    ncols = len(kws) * 128
                    for h in range(HPC):
                        p0 = (h % 2) * 64
                        hf = h // 2
                        stp = st_psum.tile([128, 3 * 128], F32,
                                           name=f"st{w}_{h}", tag="st")
                        rhs_q = qts[c][hf][p0:p0 + 64,
                                           wi * 128:(wi + 1) * 128]
                        for j, kw in enumerate(kws):
                            lhs_k = kts[kw // WPC][hf][
                                p0:p0 + 64,
                                (kw % WPC) * 128:(kw % WPC + 1) * 128,
                            ]
                            nc.tensor.matmul(
                                stp[:, j * 128:(j + 1) * 128], lhs_k, rhs_q,
                                start=True, stop=True,
                            )
                        pt = pt_pool.tile([128, 3 * 128], R,
                                          name=f"pt{w}_{h}", tag="pt")
                        nc.scalar.activation(pt[:, :ncols], stp[:, :ncols],
                                             EXP, bias=0.0, scale=SCALE)
                        op = o_psum.tile([128, 66], F32, name=f"o{w}_{h}",
                                         tag="o")
                        for j, kw in enumerate(kws):
                            rhs_v = vts[kw // WPC][kw % WPC][
                                :, h * 66:(h + 1) * 66]
                            nc.tensor.matmul(
                                op, pt[:, j * 128:(j + 1) * 128],
                                rhs_v,
                                start=(j == 0), stop=(j == len(kws) - 1),
                            )
                        rt = rcp_pool.tile([128, 1], F32, name=f"r{w}_{h}",
                                           tag="r")
                        nc.vector.reciprocal(rt, op[:, 64:65])
                        nc.vector.tensor_scalar_mul(
                            osb[:, h * 64:(h + 1) * 64], op[:, 0:64], rt)
                    nc.sync.dma_start(out_d[w * 128:(w + 1) * 128, :], osb)

            phase1(0)
            for c in range(1, NCH):
                phase1(c)
                attn(c - 1)
            attn(NCH - 1)

    nc.compile()
    return nc


def get_nc():
    if not _NC_CACHE:
        _NC_CACHE.append(_build_nc())
    return _NC_CACHE[0]


def _in_maps(x, w_qkv):
    maps = []
    for c in range(8):
        b, g = c // 2, c % 2
        maps.append({
            "x": np.ascontiguousarray(x[b]),
            "wq": np.ascontiguousarray(w_qkv[:, g * F:(g + 1) * F]),
            "wk": np.ascontiguousarray(w_qkv[:, D + g * F:D + (g + 1) * F]),
            "wv": np.ascontiguousarray(
                w_qkv[:, 2 * D + g * F:2 * D + (g + 1) * F]),
        })
    return maps


def kernel(x, w_qkv, **run_kwargs):
    x = np.asarray(x, dtype=np.float32)
    w_qkv = np.asarray(w_qkv, dtype=np.float32)
    nc = get_nc()
    res = run_bass_kernel_spmd(nc, _in_maps(x, w_qkv), list(range(8)),
                               **run_kwargs)
    out = np.empty((B, L, D), dtype=np.float32)
    for c in range(8):
        b, g = c // 2, c % 2
        out[b, :, g * F:(g + 1) * F] = res.results[c]["out"]
    if run_kwargs:
        kernel.last_results = res
    return out
